# revision 6
# baseline (speedup 1.0000x reference)
"""Causal attention (naive double-normalize == causal softmax) on 8 TRN2 cores.

Sharding:
  - Q rows interleaved: core i owns global rows {8l+i} -> uniform causal work.
  - K/V rows contiguous: core i projects rows [512i, 512(i+1)), AllGathers.

Plan R pipeline (serial on PE, collectives fully hidden):
  1. KT proj (fp16 matmuls) -> e3m4 stage -> bounce -> AG_K (1MB/rank).
  2. kt_all [128, 8, CC, 512] e3m4 preloaded to SBUF via the Scalar DMA
     queue the moment AG_K lands (Sync queue stays on weight streams).
  3. V proj (fp16) -> fp16 bounce -> AG_V (2MB/rank) - hidden under QT proj.
  4. QT proj -> qt_sb e3m4.
  5. scores: e3m4 x e3m4 matmuls from SBUF-resident kt_all/qt_sb (zero DMA
     during the scores phase -> no contention with AG_V tail).
  6. exp (scale=1/sqrt(d)) -> P fp16, causal mask on the 16-col diagonal
     straddle, rowsum via ones-matmul.
  7. AV: P fp16 x V fp16 streamed from v_ag; unscaled PSUM copy frees the
     bank; final scale by reciprocal rowsum.

The math: reference does softmax -> tril -> renormalize; the unmasked
normalizer cancels exactly, leaving causal softmax. exp stays in fp32/fp16
range without max-subtraction (max scaled score ~5.2 -> p <= ~170).
Numerics (CPU-simulated): rel err ~6.4e-3 vs fp32 reference.
"""

import math

import numpy as np

D = 2048          # d_in == d_out
CC = D // 128     # contraction chunks (16)
DT = D // 128     # output d tiles (16)
N_CORES = 8

_BUILT = {}


def _build(S):
    import concourse.bacc as bacc
    import concourse.mybir as mybir
    import concourse.tile as tile

    f32 = mybir.dt.float32
    f16 = mybir.dt.float16
    f8 = mybir.dt.float8e3
    ML = S // N_CORES          # local q rows per core (512)
    NH = ML // 128             # output row tiles per core (4)
    NJ = S // 128              # key tiles (32)
    KTR = ML // 128            # key tiles per rank (4)
    SCALE = 1.0 / math.sqrt(D)
    EXP = mybir.ActivationFunctionType.Exp
    CPY = mybir.ActivationFunctionType.Copy
    RG = [list(range(N_CORES))]

    nc = bacc.Bacc("TRN2", target_bir_lowering=False)

    xq = nc.declare_dram_parameter("xq", [128, CC, ML], f16, isOutput=False)
    xkv = nc.declare_dram_parameter("xkv", [128, CC, ML], f16, isOutput=False)
    wq = nc.declare_dram_parameter("wq", [DT, 128, CC, 128], f16, isOutput=False)
    wk = nc.declare_dram_parameter("wk", [DT, 128, CC, 128], f16, isOutput=False)
    wv = nc.declare_dram_parameter("wv", [8, 128, CC, 256], f16, isOutput=False)
    maskp = nc.declare_dram_parameter("mask", [128, 16], f16, isOutput=False)
    out = nc.declare_dram_parameter("out", [ML, D], f32, isOutput=True)

    with tile.TileContext(nc) as tc:
        with (
            tc.tile_pool(name="const", bufs=1) as const,
            tc.tile_pool(name="dram", bufs=1, space="DRAM") as dram,
        ):
            qt_sb = const.tile([128, CC, ML], f8)
            kt_all = const.tile([128, N_CORES, CC, ML], f8)
            p_all = const.tile([128, NJ, ML], f16)
            mask_sb = const.tile([128, 16], f16)
            ones_sb = const.tile([128, 1], f16)
            one1_sb = const.tile([1, 1], f32)
            rs_sb = const.tile([1, ML], f32)
            rin_sb = const.tile([128, NH], f32)
            recip_sb = const.tile([128, NH], f32)
            warm_sb = const.tile([1, 1], f32)

            kt_bounce = dram.tile([128, CC, ML], f8)
            kt_ag = dram.tile([N_CORES * 128, CC, ML], f8, addr_space="Shared")
            v_bounce = dram.tile([ML, D], f16)
            v_ag = dram.tile([S, D], f16, addr_space="Shared")

            # ============ projections ============
            with (
                tc.tile_pool(name="px", bufs=1) as px,
                tc.tile_pool(name="wkstream", bufs=4) as wkstream,
                tc.tile_pool(name="wvhold", bufs=3) as wvhold,
                tc.tile_pool(name="wqstream", bufs=4) as wqstream,
                tc.tile_pool(name="stage", bufs=4) as stage,
                tc.tile_pool(name="proj_ps", bufs=4, space="PSUM") as proj_ps,
            ):
                xkv_sb = px.tile([128, CC, ML], f16)
                xq_sb = px.tile([128, CC, ML], f16)
                nc.sync.dma_start(out=xkv_sb[:, 0:4, :], in_=xkv[:, 0:4, :])
                nc.sync.dma_start(out=xkv_sb[:, 4:CC, :], in_=xkv[:, 4:CC, :])
                nc.sync.dma_start(out=mask_sb[:], in_=maskp[:])
                nc.vector.memset(ones_sb[:], 1.0)
                nc.vector.memset(one1_sb[:], 1.0)
                # Load the Exp activation table during PE warmup, not on the
                # first score tile (critical path).
                nc.scalar.activation(
                    out=warm_sb[:], in_=one1_sb[:], func=EXP,
                )

                # ---- KT projection -> e3m4 bounce -> AG_K ----
                with tc.spectator_scope("ktproj"):
                    for dt in range(DT):
                        w = wkstream.tile([128, CC, 128], f16, tag="wk")
                        nc.sync.dma_start(out=w[:], in_=wk[dt])
                        ps = proj_ps.tile([128, ML], f32, tag="proj")
                        for c in range(CC):
                            nc.tensor.matmul(
                                out=ps[:], lhsT=w[:, c, :], rhs=xkv_sb[:, c, :],
                                start=(c == 0), stop=(c == CC - 1),
                            )
                        st = stage.tile([128, ML], f8, tag="kst")
                        nc.vector.tensor_copy(out=st[:], in_=ps[:])
                        nc.sync.dma_start(out=kt_bounce[:, dt, :], in_=st[:])
                    nc.gpsimd.collective_compute(
                        "AllGather", mybir.AluOpType.bypass,
                        replica_groups=RG,
                        ins=[kt_bounce[:].opt()], outs=[kt_ag[:].opt()],
                    )
                    # Preload the gathered KT into SBUF on the Scalar DMA
                    # queue (Sync stays on weight streams). Rank-ordered so
                    # the scores phase can chase the preload.
                    for r in range(N_CORES):
                        nc.scalar.dma_start(
                            out=kt_all[:, r], in_=kt_ag[128 * r:128 * (r + 1)]
                        )

                # ---- V projection -> fp16 bounce -> AG_V ----
                nc.sync.dma_start(out=xq_sb[:], in_=xq[:])
                with tc.spectator_scope("vproj"):
                    for wc in range(8):
                        wvt = wvhold.tile([128, CC, 256], f16, tag="wv")
                        nc.sync.dma_start(out=wvt[:], in_=wv[wc])
                        for nt in range(NH):
                            ps = proj_ps.tile([128, ML], f32, tag="proj")
                            for c in range(CC):
                                nc.tensor.matmul(
                                    out=ps[:, 0:256],
                                    lhsT=xkv_sb[:, c, 128 * nt:128 * (nt + 1)],
                                    rhs=wvt[:, c, :],
                                    start=(c == 0), stop=(c == CC - 1),
                                )
                            st = stage.tile([128, 256], f16, tag="vst")
                            nc.vector.tensor_copy(out=st[:], in_=ps[:, 0:256])
                            nc.sync.dma_start(
                                out=v_bounce[128 * nt:128 * (nt + 1),
                                             256 * wc:256 * (wc + 1)],
                                in_=st[:],
                            )
                    nc.gpsimd.collective_compute(
                        "AllGather", mybir.AluOpType.bypass,
                        replica_groups=RG,
                        ins=[v_bounce[:].opt()], outs=[v_ag[:].opt()],
                    )

                # ---- QT projection -> qt_sb e3m4 ----
                with tc.spectator_scope("qtproj"):
                    for dt in range(DT):
                        wqt = wqstream.tile([128, CC, 128], f16, tag="wq")
                        nc.sync.dma_start(out=wqt[:], in_=wq[dt])
                        ps = proj_ps.tile([128, ML], f32, tag="proj")
                        for c in range(CC):
                            nc.tensor.matmul(
                                out=ps[:], lhsT=wqt[:, c, :], rhs=xq_sb[:, c, :],
                                start=(c == 0), stop=(c == CC - 1),
                            )
                        nc.vector.tensor_copy(out=qt_sb[:, dt, :], in_=ps[:])

            # ============ attention ============
            with (
                tc.tile_pool(name="vstream", bufs=6) as vstream,
                tc.tile_pool(name="avstage", bufs=8) as avstage,
                tc.tile_pool(name="outp", bufs=4) as outp,
                tc.tile_pool(name="st_ps", bufs=2, space="PSUM") as st_ps,
                tc.tile_pool(name="rs_ps", bufs=1, space="PSUM") as rs_ps,
                tc.tile_pool(name="av_ps", bufs=1, space="PSUM") as av_ps,
                tc.tile_pool(name="tp_ps", bufs=1, space="PSUM") as tp_ps,
            ):
                rs = rs_ps.tile([1, ML], f32)
                with tc.spectator_scope("scores"):
                    for j in range(NJ):
                        r, n0 = j // KTR, 128 * (j % KTR)
                        m0 = 16 * j
                        ps = st_ps.tile([128, ML], f32, tag="st")
                        for c in range(CC):
                            nc.tensor.matmul(
                                out=ps[:, m0:ML],
                                lhsT=kt_all[:, r, c, n0:n0 + 128],
                                rhs=qt_sb[:, c, m0:ML],
                                start=(c == 0), stop=(c == CC - 1),
                            )
                        pj = p_all[:, j, :]
                        nc.scalar.activation(
                            out=pj[:, m0:ML], in_=ps[:, m0:ML], func=EXP,
                            scale=SCALE,
                        )
                        nc.vector.tensor_tensor(
                            out=pj[:, m0:m0 + 16], in0=pj[:, m0:m0 + 16],
                            in1=mask_sb[:], op=mybir.AluOpType.mult,
                        )
                        g0 = 128 * (j // 8)
                        if m0 > g0:
                            nc.vector.memset(pj[:, g0:m0], 0.0)
                        nc.tensor.matmul(
                            out=rs[0:1, m0:ML], lhsT=ones_sb[:],
                            rhs=pj[:, m0:ML],
                            start=(j == 0), stop=(j == NJ - 1),
                        )

                with tc.spectator_scope("renorm"):
                    nc.vector.tensor_copy(out=rs_sb[:], in_=rs[:])
                    for h in range(NH):
                        tp = tp_ps.tile([128, 1], f32, tag="tp")
                        nc.tensor.matmul(
                            out=tp[:], lhsT=rs_sb[0:1, 128 * h:128 * (h + 1)],
                            rhs=one1_sb[:], start=True, stop=True,
                        )
                        nc.vector.tensor_copy(out=rin_sb[:, h:h + 1], in_=tp[:])
                    nc.vector.reciprocal(out=recip_sb[:], in_=rin_sb[:])

                with tc.spectator_scope("av"):
                    for cs in range(4):
                        av = [
                            av_ps.tile([128, 512], f32, tag=f"av{h}", name=f"av{h}_{cs}")
                            for h in range(NH)
                        ]
                        for t in range((NJ + 3) // 4):
                            vt = vstream.tile([128, 4, 512], f16, tag="v")
                            nc.sync.dma_start(
                                out=vt[:],
                                in_=v_ag[512 * t:512 * (t + 1), 512 * cs:512 * (cs + 1)]
                                .rearrange("(jj p) n -> p jj n", p=128),
                            )
                            for jj in range(4):
                                j = 4 * t + jj
                                for h in range(j // 8, NH):
                                    nc.tensor.matmul(
                                        out=av[h][:],
                                        lhsT=p_all[:, j, 128 * h:128 * (h + 1)],
                                        rhs=vt[:, jj, :],
                                        start=(j == 0),
                                        stop=(j == min(8 * (h + 1), NJ) - 1),
                                    )
                        for h in range(NH):
                            # Unscaled copy frees the PSUM bank immediately so
                            # the next cs never waits on the reciprocal chain.
                            stg = avstage.tile(
                                [128, 512], f32, tag="avs", name=f"avs{h}_{cs}"
                            )
                            nc.vector.tensor_copy(out=stg[:], in_=av[h][:])
                            ob = outp.tile([128, 512], f32, tag="out")
                            nc.scalar.activation(
                                out=ob[:], in_=stg[:], func=CPY,
                                scale=recip_sb[:, h:h + 1],
                            )
                            nc.sync.dma_start(
                                out=out[128 * h:128 * (h + 1), 512 * cs:512 * (cs + 1)],
                                in_=ob[:],
                            )

    nc.finalize()
    return nc


def _prep_inputs(x, Wq, Wk, Wv, S):
    f16 = np.float16
    ML = S // N_CORES

    def shuf_w(W):
        # [dt, p, c, j] layout: element = W[128c+p, 128dt+j]
        return np.ascontiguousarray(
            W.reshape(CC, 128, DT, 128).transpose(2, 1, 0, 3)
        ).astype(f16)

    wq_h = shuf_w(Wq)
    wk_h = shuf_w(Wk)
    # wv [wc, p, c, j]: element = Wv[128c+p, 256wc+j]
    wv_h = np.ascontiguousarray(
        Wv.reshape(CC, 128, 8, 256).transpose(2, 1, 0, 3)
    ).astype(f16)

    def shuf_x(rows):
        # rows [ML, D] -> [p, c, m] with element = rows[m, 128c+p]
        return np.ascontiguousarray(rows.reshape(ML, CC, 128).transpose(2, 1, 0)).astype(f16)

    in_maps = []
    for i in range(N_CORES):
        mask = (np.arange(128)[:, None] <= 8 * np.arange(16)[None, :] + i).astype(f16)
        in_maps.append({
            "xq": shuf_x(x[i::N_CORES]),
            "xkv": shuf_x(x[ML * i:ML * (i + 1)]),
            "wq": wq_h, "wk": wk_h, "wv": wv_h,
            "mask": mask,
        })
    return in_maps


def run(x, Wq, Wk, Wv, S, trace=False, trace_cores=None):
    from concourse.bass_utils import run_bass_kernel_spmd

    if S not in _BUILT:
        _BUILT[S] = _build(S)
    nc = _BUILT[S]
    in_maps = _prep_inputs(x, Wq, Wk, Wv, S)
    res = run_bass_kernel_spmd(
        nc, in_maps, list(range(N_CORES)), trace=trace, trace_cores=trace_cores
    )
    outs = [res.results[i]["out"] for i in range(N_CORES)]
    full = np.stack(outs, axis=1).reshape(S, D).astype(np.float32)
    return full, res


def kernel(x, Wq, Wk, Wv):
    x = np.asarray(x, dtype=np.float32)
    Wq = np.asarray(Wq, dtype=np.float32)
    Wk = np.asarray(Wk, dtype=np.float32)
    Wv = np.asarray(Wv, dtype=np.float32)
    full, _ = run(x, Wq, Wk, Wv, x.shape[0])
    return full


# revision 7
# speedup vs baseline: 1.0499x; 1.0499x over previous
"""Causal attention (naive double-normalize == causal softmax) on 8 TRN2 cores.

Sharding:
  - Q rows interleaved: core i owns global rows {8l+i} -> uniform causal work.
  - K/V rows contiguous: core i projects rows [512i, 512(i+1)), AllGathers.

Schedule (v3): the AllGathers saturate HBM, so any phase that overlaps an
AG window must be DMA-free (operands SBUF-resident):

  phase                DMA in flight                 collective
  KT proj   19-86us    wk roll (sync) + wq prefetch (scalar)
  QT proj   86-153     none (wq+xq resident)         AG_K (e3m4, hidden)
  V proj   153-220     wv roll + kt_all preload
  scores   220-298     none (kt_all+qt_sb resident)  AG_V (fp16, hidden)
  AV       300-385     v_ag stream (full speed)

Dtypes: fp16 weights/x/V/P everywhere; KT/QT stored e3m4 (halves AG_K and
keeps all of KT SBUF-resident for the scores phase); scores matmul is
e3m4 x e3m4 (runs at bf16 speed); PSUM always fp32.

The math: reference does softmax -> tril -> renormalize; the unmasked
normalizer cancels exactly, leaving causal softmax. exp stays in range
without max-subtraction (max scaled score ~5.2 -> p <= ~170, fp16-safe).
Numerics (CPU-simulated): rel err ~6.4e-3 vs fp32 reference.
"""

import math

import numpy as np

D = 2048          # d_in == d_out
CC = D // 128     # contraction chunks (16)
DT = D // 128     # output d tiles (16)
N_CORES = 8

_BUILT = {}


def _build(S):
    import concourse.bacc as bacc
    import concourse.mybir as mybir
    import concourse.tile as tile

    f32 = mybir.dt.float32
    f16 = mybir.dt.float16
    f8 = mybir.dt.float8e3
    ML = S // N_CORES          # local q rows per core (512)
    NH = ML // 128             # output row tiles per core (4)
    NJ = S // 128              # key tiles (32)
    KTR = ML // 128            # key tiles per rank (4)
    SCALE = 1.0 / math.sqrt(D)
    EXP = mybir.ActivationFunctionType.Exp
    CPY = mybir.ActivationFunctionType.Copy
    RG = [list(range(N_CORES))]
    WQ_PRE = 12                # wq tiles prefetched via the scalar queue

    nc = bacc.Bacc("TRN2", target_bir_lowering=False)

    xq = nc.declare_dram_parameter("xq", [128, CC, ML], f16, isOutput=False)
    xkv = nc.declare_dram_parameter("xkv", [128, CC, ML], f16, isOutput=False)
    wq = nc.declare_dram_parameter("wq", [DT, 128, CC, 128], f16, isOutput=False)
    wk = nc.declare_dram_parameter("wk", [DT, 128, CC, 128], f16, isOutput=False)
    wv = nc.declare_dram_parameter("wv", [8, 128, CC, 256], f16, isOutput=False)
    maskp = nc.declare_dram_parameter("mask", [128, 16], f16, isOutput=False)
    out = nc.declare_dram_parameter("out", [ML, D], f32, isOutput=True)

    with tile.TileContext(nc) as tc:
        with (
            tc.tile_pool(name="const", bufs=1) as const,
            tc.tile_pool(name="dram", bufs=1, space="DRAM") as dram,
        ):
            qt_sb = const.tile([128, CC, ML], f8)
            kt_all = const.tile([128, N_CORES, CC, ML], f8)
            p_all = const.tile([128, NJ, ML], f16)
            mask_sb = const.tile([128, 16], f16)
            ones_sb = const.tile([128, 1], f16)
            one1_sb = const.tile([1, 1], f32)
            rs_sb = const.tile([1, ML], f32)
            rin_sb = const.tile([128, NH], f32)
            recip_sb = const.tile([128, NH], f32)
            warm_sb = const.tile([1, 1], f32)

            kt_bounce = dram.tile([128, CC, ML], f8)
            kt_ag = dram.tile([N_CORES * 128, CC, ML], f8, addr_space="Shared")
            v_bounce = dram.tile([ML, D], f16)
            v_ag = dram.tile([S, D], f16, addr_space="Shared")

            # ============ projections ============
            with (
                tc.tile_pool(name="px", bufs=1) as px,
                tc.tile_pool(name="wkstream", bufs=2) as wkstream,
                tc.tile_pool(name="stage", bufs=4) as stage,
                tc.tile_pool(name="proj_ps", bufs=4, space="PSUM") as proj_ps,
            ):
                xkv_a = px.tile([128, 4, ML], f16)
                xkv_b = px.tile([128, CC - 4, ML], f16)
                xq_sb = px.tile([128, CC, ML], f16)

                def xkv_c(c):
                    return xkv_a[:, c, :] if c < 4 else xkv_b[:, c - 4, :]

                with tc.tile_pool(name="wqhold", bufs=WQ_PRE) as wqhold:
                    wq_tiles = [
                        wqhold.tile([128, CC, 128], f16, tag="wq", name=f"wq{dt}")
                        for dt in range(DT)
                    ]
                    # Sync queue: x first, then the wk roll below.
                    nc.sync.dma_start(out=xkv_a[:], in_=xkv[:, 0:4, :])
                    nc.sync.dma_start(out=xkv_b[:], in_=xkv[:, 4:CC, :])
                    nc.sync.dma_start(out=xq_sb[:], in_=xq[:])
                    nc.sync.dma_start(out=mask_sb[:], in_=maskp[:])
                    nc.vector.memset(ones_sb[:], 1.0)
                    nc.vector.memset(one1_sb[:], 1.0)
                    # Exp table load during PE warmup + wq prefetch on the
                    # scalar DMA queue (sync stays on wk/bounce).
                    nc.scalar.activation(out=warm_sb[:], in_=one1_sb[:], func=EXP)
                    for dt in range(WQ_PRE):
                        nc.scalar.dma_start(out=wq_tiles[dt][:], in_=wq[dt])

                    # ---- KT projection -> e3m4 bounce -> AG_K ----
                    with tc.spectator_scope("ktproj"):
                        for dt in range(DT):
                            w = wkstream.tile([128, CC, 128], f16, tag="wk")
                            nc.sync.dma_start(out=w[:], in_=wk[dt])
                            ps = proj_ps.tile([128, ML], f32, tag="proj")
                            for c in range(CC):
                                nc.tensor.matmul(
                                    out=ps[:], lhsT=w[:, c, :], rhs=xkv_c(c),
                                    start=(c == 0), stop=(c == CC - 1),
                                )
                            st = stage.tile([128, ML], f8, tag="kst")
                            nc.vector.tensor_copy(out=st[:], in_=ps[:])
                            nc.sync.dma_start(out=kt_bounce[:, dt, :], in_=st[:])
                        nc.gpsimd.collective_compute(
                            "AllGather", mybir.AluOpType.bypass,
                            replica_groups=RG,
                            ins=[kt_bounce[:].opt()], outs=[kt_ag[:].opt()],
                        )
                        # Gathered-KT preload (scalar queue, rank-ordered) --
                        # fires the moment AG_K lands, runs under V proj.
                        for r in range(N_CORES):
                            nc.scalar.dma_start(
                                out=kt_all[:, r],
                                in_=kt_ag[128 * r:128 * (r + 1)],
                            )

                    # ---- QT projection (DMA-free, hides AG_K) ----
                    with tc.spectator_scope("qtproj"):
                        for dt in range(DT):
                            if dt >= WQ_PRE:
                                nc.sync.dma_start(
                                    out=wq_tiles[dt][:], in_=wq[dt]
                                )
                            ps = proj_ps.tile([128, ML], f32, tag="proj")
                            for c in range(CC):
                                nc.tensor.matmul(
                                    out=ps[:], lhsT=wq_tiles[dt][:, c, :],
                                    rhs=xq_sb[:, c, :],
                                    start=(c == 0), stop=(c == CC - 1),
                                )
                            nc.vector.tensor_copy(out=qt_sb[:, dt, :], in_=ps[:])

                # ---- V projection -> fp16 bounce -> AG_V ----
                with (
                    tc.tile_pool(name="wvhold", bufs=3) as wvhold,
                    tc.spectator_scope("vproj"),
                ):
                    for wc in range(8):
                        wvt = wvhold.tile([128, CC, 256], f16, tag="wv")
                        nc.sync.dma_start(out=wvt[:], in_=wv[wc])
                        for nt in range(NH):
                            ps = proj_ps.tile([128, ML], f32, tag="proj")
                            for c in range(CC):
                                nc.tensor.matmul(
                                    out=ps[:, 0:256],
                                    lhsT=xkv_c(c)[:, 128 * nt:128 * (nt + 1)],
                                    rhs=wvt[:, c, :],
                                    start=(c == 0), stop=(c == CC - 1),
                                )
                            st = stage.tile([128, 256], f16, tag="vst")
                            nc.vector.tensor_copy(out=st[:], in_=ps[:, 0:256])
                            nc.sync.dma_start(
                                out=v_bounce[128 * nt:128 * (nt + 1),
                                             256 * wc:256 * (wc + 1)],
                                in_=st[:],
                            )
                    nc.gpsimd.collective_compute(
                        "AllGather", mybir.AluOpType.bypass,
                        replica_groups=RG,
                        ins=[v_bounce[:].opt()], outs=[v_ag[:].opt()],
                    )

            # ============ attention ============
            with (
                tc.tile_pool(name="vstream", bufs=6) as vstream,
                tc.tile_pool(name="avstage", bufs=8) as avstage,
                tc.tile_pool(name="outp", bufs=4) as outp,
                tc.tile_pool(name="st_ps", bufs=2, space="PSUM") as st_ps,
                tc.tile_pool(name="rs_ps", bufs=1, space="PSUM") as rs_ps,
                tc.tile_pool(name="av_ps", bufs=1, space="PSUM") as av_ps,
                tc.tile_pool(name="tp_ps", bufs=1, space="PSUM") as tp_ps,
            ):
                rs = rs_ps.tile([1, ML], f32)
                # scores: DMA-free (kt_all + qt_sb resident), hides AG_V
                with tc.spectator_scope("scores"):
                    for j in range(NJ):
                        r, n0 = j // KTR, 128 * (j % KTR)
                        m0 = 16 * j
                        ps = st_ps.tile([128, ML], f32, tag="st")
                        for c in range(CC):
                            nc.tensor.matmul(
                                out=ps[:, m0:ML],
                                lhsT=kt_all[:, r, c, n0:n0 + 128],
                                rhs=qt_sb[:, c, m0:ML],
                                start=(c == 0), stop=(c == CC - 1),
                            )
                        pj = p_all[:, j, :]
                        nc.scalar.activation(
                            out=pj[:, m0:ML], in_=ps[:, m0:ML], func=EXP,
                            scale=SCALE,
                        )
                        nc.vector.tensor_tensor(
                            out=pj[:, m0:m0 + 16], in0=pj[:, m0:m0 + 16],
                            in1=mask_sb[:], op=mybir.AluOpType.mult,
                        )
                        g0 = 128 * (j // 8)
                        if m0 > g0:
                            nc.vector.memset(pj[:, g0:m0], 0.0)
                        nc.tensor.matmul(
                            out=rs[0:1, m0:ML], lhsT=ones_sb[:],
                            rhs=pj[:, m0:ML],
                            start=(j == 0), stop=(j == NJ - 1),
                        )

                with tc.spectator_scope("renorm"):
                    nc.vector.tensor_copy(out=rs_sb[:], in_=rs[:])
                    for h in range(NH):
                        tp = tp_ps.tile([128, 1], f32, tag="tp")
                        nc.tensor.matmul(
                            out=tp[:], lhsT=rs_sb[0:1, 128 * h:128 * (h + 1)],
                            rhs=one1_sb[:], start=True, stop=True,
                        )
                        nc.vector.tensor_copy(out=rin_sb[:, h:h + 1], in_=tp[:])
                    nc.vector.reciprocal(out=recip_sb[:], in_=rin_sb[:])

                with tc.spectator_scope("av"):
                    for cs in range(4):
                        av = [
                            av_ps.tile([128, 512], f32, tag=f"av{h}", name=f"av{h}_{cs}")
                            for h in range(NH)
                        ]
                        for t in range((NJ + 3) // 4):
                            vt = vstream.tile([128, 4, 512], f16, tag="v")
                            nc.sync.dma_start(
                                out=vt[:],
                                in_=v_ag[512 * t:512 * (t + 1), 512 * cs:512 * (cs + 1)]
                                .rearrange("(jj p) n -> p jj n", p=128),
                            )
                            for jj in range(4):
                                j = 4 * t + jj
                                for h in range(j // 8, NH):
                                    nc.tensor.matmul(
                                        out=av[h][:],
                                        lhsT=p_all[:, j, 128 * h:128 * (h + 1)],
                                        rhs=vt[:, jj, :],
                                        start=(j == 0),
                                        stop=(j == min(8 * (h + 1), NJ) - 1),
                                    )
                        for h in range(NH):
                            # Unscaled copy frees the PSUM bank immediately so
                            # the next cs never waits on the reciprocal chain.
                            stg = avstage.tile(
                                [128, 512], f32, tag="avs", name=f"avs{h}_{cs}"
                            )
                            nc.vector.tensor_copy(out=stg[:], in_=av[h][:])
                            ob = outp.tile([128, 512], f32, tag="out")
                            nc.scalar.activation(
                                out=ob[:], in_=stg[:], func=CPY,
                                scale=recip_sb[:, h:h + 1],
                            )
                            nc.sync.dma_start(
                                out=out[128 * h:128 * (h + 1), 512 * cs:512 * (cs + 1)],
                                in_=ob[:],
                            )

    nc.finalize()
    return nc


def _prep_inputs(x, Wq, Wk, Wv, S):
    f16 = np.float16
    ML = S // N_CORES

    def shuf_w(W):
        # [dt, p, c, j] layout: element = W[128c+p, 128dt+j]
        return np.ascontiguousarray(
            W.reshape(CC, 128, DT, 128).transpose(2, 1, 0, 3)
        ).astype(f16)

    wq_h = shuf_w(Wq)
    wk_h = shuf_w(Wk)
    # wv [wc, p, c, j]: element = Wv[128c+p, 256wc+j]
    wv_h = np.ascontiguousarray(
        Wv.reshape(CC, 128, 8, 256).transpose(2, 1, 0, 3)
    ).astype(f16)

    def shuf_x(rows):
        # rows [ML, D] -> [p, c, m] with element = rows[m, 128c+p]
        return np.ascontiguousarray(rows.reshape(ML, CC, 128).transpose(2, 1, 0)).astype(f16)

    in_maps = []
    for i in range(N_CORES):
        mask = (np.arange(128)[:, None] <= 8 * np.arange(16)[None, :] + i).astype(f16)
        in_maps.append({
            "xq": shuf_x(x[i::N_CORES]),
            "xkv": shuf_x(x[ML * i:ML * (i + 1)]),
            "wq": wq_h, "wk": wk_h, "wv": wv_h,
            "mask": mask,
        })
    return in_maps


def run(x, Wq, Wk, Wv, S, trace=False, trace_cores=None):
    from concourse.bass_utils import run_bass_kernel_spmd

    if S not in _BUILT:
        _BUILT[S] = _build(S)
    nc = _BUILT[S]
    in_maps = _prep_inputs(x, Wq, Wk, Wv, S)
    res = run_bass_kernel_spmd(
        nc, in_maps, list(range(N_CORES)), trace=trace, trace_cores=trace_cores
    )
    outs = [res.results[i]["out"] for i in range(N_CORES)]
    full = np.stack(outs, axis=1).reshape(S, D).astype(np.float32)
    return full, res


def kernel(x, Wq, Wk, Wv):
    x = np.asarray(x, dtype=np.float32)
    Wq = np.asarray(Wq, dtype=np.float32)
    Wk = np.asarray(Wk, dtype=np.float32)
    Wv = np.asarray(Wv, dtype=np.float32)
    full, _ = run(x, Wq, Wk, Wv, x.shape[0])
    return full
